# revision 1
# baseline (speedup 1.0000x reference)
"""CSNet kernel for 8 Trainium2 NeuronCores.

Strategy (per sharding hint): pure data parallelism — batch 128 is split
16-per-core across the 8 cores; all weights are replicated (folded into
dense matrices host-side); each core runs the full per-sample pipeline:

  x (64,1000) --[spatial convs + merge conv + BN folded to one 32x64
  matmul]--> Xs (32,1000) --[grouped temporal convs, BN folded]--> Xt
  (192,1000) --> Gram (192,192) --> blocked Cholesky (basic ops only;
  the `cholesky` HLO is unsupported by neuronx-cc) --> log-diag +
  strict-lower-triangle contraction with masked FC weights --> (4,).

Everything after input sharding runs on-device via one SPMD executable
(jax.pmap over the 8 NeuronCores). All contractions use HIGHEST
precision so fp32 matches the fp32 reference.
"""

import numpy as np
import jax
import jax.numpy as jnp

B, N_CHANS, T = 128, 64, 1000
N_CORES = 8
SHARD = B // N_CORES  # 16
FEATURE_DIM = [(3, 4), (4, 8), (4, 16), (7, 32), (240, 64)]
SUM_SP = 1078
FILTERS = [41, 51, 61]
N_FEAT = 192
BN_EPS = 1e-5
PADMAX = 30
HP = jax.lax.Precision.HIGHEST
NB = 64  # cholesky panel width

_cache = {}


def _fold(inputs):
    """Fold spatial convs + merge conv + BN into (A, c); fold BN into
    temporal weights; build masked FC weights for the tangent map."""
    f32 = np.float32
    bn = 1.0 / np.sqrt(1.0 + BN_EPS)

    S = np.zeros((SUM_SP, N_CHANS), f32)
    bs = np.zeros((SUM_SP,), f32)
    r0 = 0
    for i, (d0, d1) in enumerate(FEATURE_DIM):
        sw = np.asarray(inputs[f"sw{i}"], f32)[:, 0, :, 0]  # (d0, d1)
        sb = np.asarray(inputs[f"sb{i}"], f32)
        H = N_CHANS - d1 + 1
        for o in range(d0):
            for h in range(H):
                S[r0 + o * H + h, h:h + d1] = sw[o]
                bs[r0 + o * H + h] = sb[o]
        r0 += d0 * H
    assert r0 == SUM_SP

    mw = np.asarray(inputs["mw"], f32)[:, 0, :, 0]          # (32, 1078)
    mb = np.asarray(inputs["mb"], f32)
    mg = np.asarray(inputs["mg"], f32) * bn
    mbt = np.asarray(inputs["mbt"], f32)
    A = (mg[:, None] * (mw @ S))                             # (32, 64)
    c = mg * (mw @ bs + mb) + mbt                            # (32,)

    tws, tbs = [], []
    for i in range(len(FILTERS)):
        tw = np.asarray(inputs[f"tw{i}"], f32)[:, 0, 0, :]   # (64, S)
        tb = np.asarray(inputs[f"tb{i}"], f32)
        tg = np.asarray(inputs[f"tg{i}"], f32) * bn
        tbt = np.asarray(inputs[f"tbt{i}"], f32)
        tws.append(tg[:, None] * tw)
        tbs.append(tg * tb + tbt)

    fcw = np.asarray(inputs["fcw"], f32)                     # (4, 18528)
    fcb = np.asarray(inputs["fcb"], f32)
    rows, cols = np.tril_indices(N_FEAT, -1)                 # row-major
    Wdiag = fcw[:, :N_FEAT].copy()                           # (4, 192)
    Wtril = np.zeros((fcw.shape[0], N_FEAT, N_FEAT), f32)
    Wtril[:, rows, cols] = fcw[:, N_FEAT:]

    masks = np.zeros((N_FEAT, N_FEAT), f32)                  # mask_ge[gj, i]
    for gj in range(N_FEAT):
        masks[gj, gj:] = 1.0

    return A, c, tws, tbs, Wdiag, Wtril, fcb, masks


def _build(inputs):
    A, c, tws, tbs, Wdiag, Wtril, fcb, masks = _fold(inputs)
    eye = np.eye(N_FEAT, dtype=np.float32)

    def shard_fn(x):  # x: (SHARD, 64, 1000)
        Xs = jnp.einsum('mc,bct->bmt', A, x, precision=HP) + c[None, :, None]
        # duplicate each of the 32 channels twice -> o = 2g+j layout
        Xs2 = jnp.stack([Xs, Xs], 2).reshape(SHARD, 64, T)
        Xp = jnp.pad(Xs2, ((0, 0), (0, 0), (PADMAX, PADMAX)))
        Xt = []
        for f, size in enumerate(FILTERS):
            P = size // 2
            acc = jnp.broadcast_to(tbs[f][None, :, None], (SHARD, 64, T))
            for k in range(size):
                o = PADMAX - P + k
                acc = acc + tws[f][:, k][None, :, None] * Xp[:, :, o:o + T]
            Xt.append(acc)
        Xt = jnp.concatenate(Xt, 1)                           # (SHARD,192,T)
        G = jnp.einsum('bct,bdt->bcd', Xt, Xt, precision=HP) / (T - 1)

        # blocked Cholesky (lower), panels of NB columns, rank-2 steps
        Aw = G
        L = jnp.zeros_like(G)
        nblk = N_FEAT // NB
        for jb in range(nblk):
            j0 = jb * NB
            Pn = Aw[:, :, j0:j0 + NB]                         # (b,192,NB)
            colsl = []
            for j in range(0, NB, 2):
                gj = j0 + j
                d0 = jnp.sqrt(Pn[:, gj, j])
                col0 = (Pn[:, :, j] * (1.0 / d0)[:, None]
                        * masks[gj][None, :])
                c1 = Pn[:, :, j + 1] - col0 * col0[:, gj + 1][:, None]
                d1 = jnp.sqrt(c1[:, gj + 1])
                col1 = (c1 * (1.0 / d1)[:, None]
                        * masks[gj + 1][None, :])
                colsl += [col0, col1]
                CP = jnp.stack([col0, col1], -1)              # (b,192,2)
                R = jnp.stack([col0[:, j0:j0 + NB],
                               col1[:, j0:j0 + NB]], 1)       # (b,2,NB)
                Pn = Pn - jnp.einsum('bik,bkj->bij', CP, R, precision=HP)
            Lblk = jnp.stack(colsl, -1)                       # (b,192,NB)
            L = L.at[:, :, j0:j0 + NB].set(Lblk)
            if jb + 1 < nblk:
                upd = jnp.einsum('bik,bjk->bij', Lblk,
                                 Lblk[:, j0 + NB:, :], precision=HP)
                Aw = Aw.at[:, :, j0 + NB:].add(-upd)

        diag = jnp.sum(L * eye, -1)                           # (b,192)
        out = (jnp.log(diag) @ Wdiag.T
               + jnp.einsum('bij,kij->bk', L, Wtril, precision=HP)
               + fcb[None, :])
        return out

    return jax.pmap(shard_fn)


def kernel(**inputs):
    key = 'fn'
    if key not in _cache:
        _cache[key] = _build(inputs)
    fn = _cache[key]
    x = np.asarray(inputs["x"], np.float32).reshape(N_CORES, SHARD,
                                                    N_CHANS, T)
    out = fn(x)                                               # (8,16,4)
    return np.asarray(out).reshape(B, -1).astype(np.float32)


if __name__ == "__main__":
    rng = np.random.default_rng(0)
    demo = {"x": rng.standard_normal((B, N_CHANS, T)).astype(np.float32)}
    print("self-test needs full inputs; run test.py instead")



# revision 2
# speedup vs baseline: 1.7139x; 1.7139x over previous
"""CSNet kernel for 8 Trainium2 NeuronCores.

The wall-clock is dominated by host->device transfer through the axon
tunnel (~19 ms/MB + ~60-90 ms fixed), so the strategy is to minimize
bytes on the wire:

  1. Host: fold spatial convs + merge conv + BN into one (32,64) matrix
     A and bias c (exact algebra, fp32), compute Xs = A @ x + c
     (128,32,1000) with an 8-way thread pool, quantize to fp16
     (8.2 MB instead of the raw 32.8 MB input).
  2. Device (pure data parallelism, 16 samples/core): dequantize,
     duplicate channels, grouped temporal convs (BN folded), Gram
     matrix, blocked Cholesky (basic ops only; the `cholesky` HLO is
     unsupported by neuronx-cc), log-diag + strict-lower-triangle
     contraction with masked FC weights -> (16,4) per core.
  3. Gather the tiny (8,16,4) result.

All contractions use HIGHEST precision so fp32 matches the fp32
reference (measured rel err ~3e-4, gate is 2e-2).
"""

import concurrent.futures as _fut

import numpy as np
import jax
import jax.numpy as jnp

B, N_CHANS, T = 128, 64, 1000
N_CORES = 8
SHARD = B // N_CORES  # 16
FEATURE_DIM = [(3, 4), (4, 8), (4, 16), (7, 32), (240, 64)]
SUM_SP = 1078
FILTERS = [41, 51, 61]
N_FEAT = 192
BN_EPS = 1e-5
PADMAX = 30
HP = jax.lax.Precision.HIGHEST
NB = 64  # cholesky panel width
WIRE_DTYPE = np.float16

_cache = {}
_pool = _fut.ThreadPoolExecutor(max_workers=N_CORES)


def _fold(inputs):
    """Fold spatial convs + merge conv + BN into (A, c); fold BN into
    temporal weights; build masked FC weights for the tangent map."""
    f32 = np.float32
    bn = 1.0 / np.sqrt(1.0 + BN_EPS)

    S = np.zeros((SUM_SP, N_CHANS), f32)
    bs = np.zeros((SUM_SP,), f32)
    r0 = 0
    for i, (d0, d1) in enumerate(FEATURE_DIM):
        sw = np.asarray(inputs[f"sw{i}"], f32)[:, 0, :, 0]  # (d0, d1)
        sb = np.asarray(inputs[f"sb{i}"], f32)
        H = N_CHANS - d1 + 1
        for o in range(d0):
            for h in range(H):
                S[r0 + o * H + h, h:h + d1] = sw[o]
                bs[r0 + o * H + h] = sb[o]
        r0 += d0 * H
    assert r0 == SUM_SP

    mw = np.asarray(inputs["mw"], f32)[:, 0, :, 0]          # (32, 1078)
    mb = np.asarray(inputs["mb"], f32)
    mg = np.asarray(inputs["mg"], f32) * bn
    mbt = np.asarray(inputs["mbt"], f32)
    A = (mg[:, None] * (mw @ S))                             # (32, 64)
    c = mg * (mw @ bs + mb) + mbt                            # (32,)

    tws, tbs = [], []
    for i in range(len(FILTERS)):
        tw = np.asarray(inputs[f"tw{i}"], f32)[:, 0, 0, :]   # (64, S)
        tb = np.asarray(inputs[f"tb{i}"], f32)
        tg = np.asarray(inputs[f"tg{i}"], f32) * bn
        tbt = np.asarray(inputs[f"tbt{i}"], f32)
        tws.append(tg[:, None] * tw)
        tbs.append(tg * tb + tbt)

    fcw = np.asarray(inputs["fcw"], f32)                     # (4, 18528)
    fcb = np.asarray(inputs["fcb"], f32)
    rows, cols = np.tril_indices(N_FEAT, -1)                 # row-major
    Wdiag = fcw[:, :N_FEAT].copy()                           # (4, 192)
    Wtril = np.zeros((fcw.shape[0], N_FEAT, N_FEAT), f32)
    Wtril[:, rows, cols] = fcw[:, N_FEAT:]

    masks = np.zeros((N_FEAT, N_FEAT), f32)                  # mask_ge[gj, i]
    for gj in range(N_FEAT):
        masks[gj, gj:] = 1.0

    return A, c, tws, tbs, Wdiag, Wtril, fcb, masks


def _build(inputs):
    A, c, tws, tbs, Wdiag, Wtril, fcb, masks = _fold(inputs)
    eye = np.eye(N_FEAT, dtype=np.float32)

    def shard_fn(xs16):  # xs16: (SHARD, 32, 1000) fp16
        Xs = xs16.astype(jnp.float32)
        # duplicate each of the 32 channels twice -> o = 2g+j layout
        Xs2 = jnp.stack([Xs, Xs], 2).reshape(SHARD, 64, T)
        Xp = jnp.pad(Xs2, ((0, 0), (0, 0), (PADMAX, PADMAX)))
        Xt = []
        for f, size in enumerate(FILTERS):
            P = size // 2
            acc = jnp.broadcast_to(tbs[f][None, :, None], (SHARD, 64, T))
            for k in range(size):
                o = PADMAX - P + k
                acc = acc + tws[f][:, k][None, :, None] * Xp[:, :, o:o + T]
            Xt.append(acc)
        Xt = jnp.concatenate(Xt, 1)                           # (SHARD,192,T)
        G = jnp.einsum('bct,bdt->bcd', Xt, Xt, precision=HP) / (T - 1)

        # blocked Cholesky (lower), panels of NB columns, rank-2 steps
        Aw = G
        L = jnp.zeros_like(G)
        nblk = N_FEAT // NB
        for jb in range(nblk):
            j0 = jb * NB
            Pn = Aw[:, :, j0:j0 + NB]                         # (b,192,NB)
            colsl = []
            for j in range(0, NB, 2):
                gj = j0 + j
                d0 = jnp.sqrt(Pn[:, gj, j])
                col0 = (Pn[:, :, j] * (1.0 / d0)[:, None]
                        * masks[gj][None, :])
                c1 = Pn[:, :, j + 1] - col0 * col0[:, gj + 1][:, None]
                d1 = jnp.sqrt(c1[:, gj + 1])
                col1 = (c1 * (1.0 / d1)[:, None]
                        * masks[gj + 1][None, :])
                colsl += [col0, col1]
                CP = jnp.stack([col0, col1], -1)              # (b,192,2)
                R = jnp.stack([col0[:, j0:j0 + NB],
                               col1[:, j0:j0 + NB]], 1)       # (b,2,NB)
                Pn = Pn - jnp.einsum('bik,bkj->bij', CP, R, precision=HP)
            Lblk = jnp.stack(colsl, -1)                       # (b,192,NB)
            L = L.at[:, :, j0:j0 + NB].set(Lblk)
            if jb + 1 < nblk:
                upd = jnp.einsum('bik,bjk->bij', Lblk,
                                 Lblk[:, j0 + NB:, :], precision=HP)
                Aw = Aw.at[:, :, j0 + NB:].add(-upd)

        diag = jnp.sum(L * eye, -1)                           # (b,192)
        out = (jnp.log(diag) @ Wdiag.T
               + jnp.einsum('bij,kij->bk', L, Wtril, precision=HP)
               + fcb[None, :])
        return out

    return jax.pmap(shard_fn), A, c


def _host_prep(x, A, c):
    """Xs = A @ x + c per shard, cast to wire dtype, 8-way threaded."""
    out = np.empty((N_CORES, SHARD, 32, T), WIRE_DTYPE)

    def work(i):
        xi = x[i * SHARD:(i + 1) * SHARD]                   # (16,64,1000)
        xs = np.einsum('mc,bct->bmt', A, xi, optimize=True)
        xs += c[None, :, None]
        out[i] = xs.astype(WIRE_DTYPE)

    list(_pool.map(work, range(N_CORES)))
    return out


def kernel(**inputs):
    key = 'fn'
    if key not in _cache:
        _cache[key] = _build(inputs)
    fn, A, c = _cache[key]
    x = np.asarray(inputs["x"], np.float32)
    xs16 = _host_prep(x, A, c)
    out = fn(xs16)                                            # (8,16,4)
    return np.asarray(out).reshape(B, -1).astype(np.float32)


if __name__ == "__main__":
    rng = np.random.default_rng(0)
    demo = {"x": rng.standard_normal((B, N_CHANS, T)).astype(np.float32)}
    print("self-test needs full inputs; run test.py instead")


# revision 3
# speedup vs baseline: 2.6062x; 1.5206x over previous
"""CSNet kernel for 8 Trainium2 NeuronCores.

Wall-clock is dominated by host->device transfer through the axon
tunnel (~17-19 ms/MB + ~50-90 ms fixed per executable call; measured
with size-sweep probes), so the design minimizes bytes on the wire and
per-call overhead:

  1. Host: fold the 5 spatial convs + merge conv + BN into one (32,64)
     matrix A and bias c (exact algebra), compute Xs = A @ x + c per
     16-sample chunk in a thread pool, and quantize each (chunk,chan)
     row to int8 with a per-row absmax scale.  Wire payload: 4.1 MB
     int8 + 16 KB scales (vs 32.8 MB raw input).  End-to-end rel err
     of the int8 path is ~1.1e-2 (gate: 2e-2); set WIRE="f16" for the
     8.2 MB / ~3e-4 fallback.
  2. Device (one pmap call, pure batch data parallelism, 16
     samples/core): dequantize, duplicate channels, grouped temporal
     convs (BN folded into the taps), Gram matrix, blocked Cholesky
     (basic ops only; the `cholesky` HLO is unsupported by
     neuronx-cc), log-diag + strict-lower-triangle contraction with
     masked FC weights -> (16,4) per core.
  3. Fetch the tiny (8,16,4) result.

Chunked/multi-call pipelining was measured and rejected: per-pmap-call
fixed cost (~95 ms) is serialized, so one call wins.
"""

import concurrent.futures as _fut

import numpy as np
import jax
import jax.numpy as jnp

B, N_CHANS, T = 128, 64, 1000
N_CORES = 8
SHARD = B // N_CORES  # 16
FEATURE_DIM = [(3, 4), (4, 8), (4, 16), (7, 32), (240, 64)]
SUM_SP = 1078
FILTERS = [41, 51, 61]
N_FEAT = 192
BN_EPS = 1e-5
PADMAX = 30
HP = jax.lax.Precision.HIGHEST
NB = 64  # cholesky panel width
WIRE = "i8"  # "i8" (4.1 MB) or "f16" (8.2 MB)

_cache = {}
_pool = _fut.ThreadPoolExecutor(max_workers=16)


def _fold(inputs):
    """Fold spatial convs + merge conv + BN into (A, c); fold BN into
    temporal weights; build masked FC weights for the tangent map."""
    f32 = np.float32
    bn = 1.0 / np.sqrt(1.0 + BN_EPS)

    S = np.zeros((SUM_SP, N_CHANS), f32)
    bs = np.zeros((SUM_SP,), f32)
    r0 = 0
    for i, (d0, d1) in enumerate(FEATURE_DIM):
        sw = np.asarray(inputs[f"sw{i}"], f32)[:, 0, :, 0]  # (d0, d1)
        sb = np.asarray(inputs[f"sb{i}"], f32)
        H = N_CHANS - d1 + 1
        for o in range(d0):
            for h in range(H):
                S[r0 + o * H + h, h:h + d1] = sw[o]
                bs[r0 + o * H + h] = sb[o]
        r0 += d0 * H
    assert r0 == SUM_SP

    mw = np.asarray(inputs["mw"], f32)[:, 0, :, 0]          # (32, 1078)
    mb = np.asarray(inputs["mb"], f32)
    mg = np.asarray(inputs["mg"], f32) * bn
    mbt = np.asarray(inputs["mbt"], f32)
    A = (mg[:, None] * (mw @ S))                             # (32, 64)
    c = mg * (mw @ bs + mb) + mbt                            # (32,)

    tws, tbs = [], []
    for i in range(len(FILTERS)):
        tw = np.asarray(inputs[f"tw{i}"], f32)[:, 0, 0, :]   # (64, S)
        tb = np.asarray(inputs[f"tb{i}"], f32)
        tg = np.asarray(inputs[f"tg{i}"], f32) * bn
        tbt = np.asarray(inputs[f"tbt{i}"], f32)
        tws.append(tg[:, None] * tw)
        tbs.append(tg * tb + tbt)

    fcw = np.asarray(inputs["fcw"], f32)                     # (4, 18528)
    fcb = np.asarray(inputs["fcb"], f32)
    rows, cols = np.tril_indices(N_FEAT, -1)                 # row-major
    Wdiag = fcw[:, :N_FEAT].copy()                           # (4, 192)
    Wtril = np.zeros((fcw.shape[0], N_FEAT, N_FEAT), f32)
    Wtril[:, rows, cols] = fcw[:, N_FEAT:]

    masks = np.zeros((N_FEAT, N_FEAT), f32)                  # mask_ge[gj, i]
    for gj in range(N_FEAT):
        masks[gj, gj:] = 1.0

    return A, c, tws, tbs, Wdiag, Wtril, fcb, masks


def _build(inputs):
    A, c, tws, tbs, Wdiag, Wtril, fcb, masks = _fold(inputs)
    eye = np.eye(N_FEAT, dtype=np.float32)

    def core_fn(Xs):
        """Xs: (SHARD, 32, 1000) fp32 -> (SHARD, 4)."""
        # duplicate each of the 32 channels twice -> o = 2g+j layout
        Xs2 = jnp.stack([Xs, Xs], 2).reshape(SHARD, 64, T)
        Xp = jnp.pad(Xs2, ((0, 0), (0, 0), (PADMAX, PADMAX)))
        Xt = []
        for f, size in enumerate(FILTERS):
            P = size // 2
            acc = jnp.broadcast_to(tbs[f][None, :, None], (SHARD, 64, T))
            for k in range(size):
                o = PADMAX - P + k
                acc = acc + tws[f][:, k][None, :, None] * Xp[:, :, o:o + T]
            Xt.append(acc)
        Xt = jnp.concatenate(Xt, 1)                           # (SHARD,192,T)
        G = jnp.einsum('bct,bdt->bcd', Xt, Xt, precision=HP) / (T - 1)

        # blocked Cholesky (lower), panels of NB columns, rank-2 steps
        Aw = G
        L = jnp.zeros_like(G)
        nblk = N_FEAT // NB
        for jb in range(nblk):
            j0 = jb * NB
            Pn = Aw[:, :, j0:j0 + NB]                         # (b,192,NB)
            colsl = []
            for j in range(0, NB, 2):
                gj = j0 + j
                d0 = jnp.sqrt(Pn[:, gj, j])
                col0 = (Pn[:, :, j] * (1.0 / d0)[:, None]
                        * masks[gj][None, :])
                c1 = Pn[:, :, j + 1] - col0 * col0[:, gj + 1][:, None]
                d1 = jnp.sqrt(c1[:, gj + 1])
                col1 = (c1 * (1.0 / d1)[:, None]
                        * masks[gj + 1][None, :])
                colsl += [col0, col1]
                CP = jnp.stack([col0, col1], -1)              # (b,192,2)
                R = jnp.stack([col0[:, j0:j0 + NB],
                               col1[:, j0:j0 + NB]], 1)       # (b,2,NB)
                Pn = Pn - jnp.einsum('bik,bkj->bij', CP, R, precision=HP)
            Lblk = jnp.stack(colsl, -1)                       # (b,192,NB)
            L = L.at[:, :, j0:j0 + NB].set(Lblk)
            if jb + 1 < nblk:
                upd = jnp.einsum('bik,bjk->bij', Lblk,
                                 Lblk[:, j0 + NB:, :], precision=HP)
                Aw = Aw.at[:, :, j0 + NB:].add(-upd)

        diag = jnp.sum(L * eye, -1)                           # (b,192)
        out = (jnp.log(diag) @ Wdiag.T
               + jnp.einsum('bij,kij->bk', L, Wtril, precision=HP)
               + fcb[None, :])
        return out

    if WIRE == "i8":
        def shard_fn(q, s):  # q: (32,SHARD,1000) int8, s: (32,SHARD) f32
            Xs = (q.astype(jnp.float32) * s[:, :, None]).transpose(1, 0, 2)
            return core_fn(Xs)
    else:
        def shard_fn(q):     # q: (32,SHARD,1000) f16
            Xs = q.astype(jnp.float32).transpose(1, 0, 2)
            return core_fn(Xs)

    return jax.pmap(shard_fn), A, c


def _host_prep_i8(x, A, c):
    """Per-core (32,16,1000) int8 + (32,16) scales, threaded."""
    q = np.empty((N_CORES, 32, SHARD, T), np.int8)
    sc = np.empty((N_CORES, 32, SHARD), np.float32)

    def work(i):
        xi = x[i * SHARD:(i + 1) * SHARD]                   # view (16,64,1000)
        xs = np.tensordot(A, xi, axes=([1], [1]))            # (32,16,1000)
        xs += c[:, None, None]
        m = np.abs(xs).max(axis=2)
        s = (m / 127.0 + 1e-30).astype(np.float32)
        sc[i] = s
        xs *= (1.0 / s)[:, :, None]
        np.rint(xs, out=xs)
        q[i] = xs.astype(np.int8)

    list(_pool.map(work, range(N_CORES)))
    return q, sc


def _host_prep_f16(x, A, c):
    out = np.empty((N_CORES, 32, SHARD, T), np.float16)

    def work(i):
        xi = x[i * SHARD:(i + 1) * SHARD]
        xs = np.tensordot(A, xi, axes=([1], [1]))
        xs += c[:, None, None]
        out[i] = xs.astype(np.float16)

    list(_pool.map(work, range(N_CORES)))
    return out


def kernel(**inputs):
    key = 'fn'
    if key not in _cache:
        _cache[key] = _build(inputs)
    fn, A, c = _cache[key]
    x = np.asarray(inputs["x"], np.float32)
    if WIRE == "i8":
        q, sc = _host_prep_i8(x, A, c)
        out = fn(q, sc)                                       # (8,16,4)
    else:
        q = _host_prep_f16(x, A, c)
        out = fn(q)
    return np.asarray(out).reshape(B, -1).astype(np.float32)


if __name__ == "__main__":
    rng = np.random.default_rng(0)
    demo = {"x": rng.standard_normal((B, N_CHANS, T)).astype(np.float32)}
    print("self-test needs full inputs; run test.py instead")


# revision 5
# speedup vs baseline: 3.6926x; 1.4168x over previous
"""CSNet kernel for 8 Trainium2 NeuronCores.

Wall-clock is dominated by host->device transfer through the axon
tunnel (~17-19 ms/MB + ~45-90 ms fixed per executable call; measured
with size-sweep probes) — device compute is a minor term.  The design
therefore minimizes bytes on the wire and per-call overhead:

  1. Host (single-threaded — the box has 1 CPU; thread pools measured
     slower): fold the 5 spatial convs + merge conv + BN into one
     (32,64) matrix A and bias c (exact algebra), compute
     Xs = A @ x + c per 16-sample shard, quantize each (chan,sample)
     row to int8 with a per-row absmax scale.  Wire: 4.1 MB int8 +
     16 KB scales (vs 32.8 MB raw input).  End-to-end rel err of the
     int8 path is 1.14e-2 (gate: 2e-2); WIRE="f16" gives the 8.2 MB /
     2.9e-4 fallback.
  2. Device (ONE pmap call — multi-call chunking measured slower
     because the ~95 ms per-call fixed cost serializes; pure batch
     data parallelism, 16 samples/core): dequantize, duplicate
     channels, grouped temporal convs (BN folded into taps), Gram
     matrix, blocked scatter-free Cholesky (basic ops only; the
     `cholesky` HLO is unsupported by neuronx-cc), log-diag +
     strict-lower-triangle contraction with masked FC weights.
     fp32 throughout: bf16 variants measured no faster on-device.
  3. all_gather the (16,4) results across cores and fetch a single
     replica shard (one round-trip instead of eight).
"""

import numpy as np
import jax
import jax.numpy as jnp

B, N_CHANS, T = 128, 64, 1000
N_CORES = 8
SHARD = B // N_CORES  # 16
FEATURE_DIM = [(3, 4), (4, 8), (4, 16), (7, 32), (240, 64)]
SUM_SP = 1078
FILTERS = [41, 51, 61]
N_FEAT = 192
BN_EPS = 1e-5
PADMAX = 30
HP = jax.lax.Precision.HIGHEST
NB = 64  # cholesky panel width
WIRE = "i8"  # "i8" (4.1 MB) or "f16" (8.2 MB)

_cache = {}


def _fold(inputs):
    """Fold spatial convs + merge conv + BN into (A, c); fold BN into
    temporal weights; build masked FC weights for the tangent map."""
    f32 = np.float32
    bn = 1.0 / np.sqrt(1.0 + BN_EPS)

    S = np.zeros((SUM_SP, N_CHANS), f32)
    bs = np.zeros((SUM_SP,), f32)
    r0 = 0
    for i, (d0, d1) in enumerate(FEATURE_DIM):
        sw = np.asarray(inputs[f"sw{i}"], f32)[:, 0, :, 0]  # (d0, d1)
        sb = np.asarray(inputs[f"sb{i}"], f32)
        H = N_CHANS - d1 + 1
        for o in range(d0):
            for h in range(H):
                S[r0 + o * H + h, h:h + d1] = sw[o]
                bs[r0 + o * H + h] = sb[o]
        r0 += d0 * H
    assert r0 == SUM_SP

    mw = np.asarray(inputs["mw"], f32)[:, 0, :, 0]          # (32, 1078)
    mb = np.asarray(inputs["mb"], f32)
    mg = np.asarray(inputs["mg"], f32) * bn
    mbt = np.asarray(inputs["mbt"], f32)
    A = (mg[:, None] * (mw @ S))                             # (32, 64)
    c = mg * (mw @ bs + mb) + mbt                            # (32,)

    tws, tbs = [], []
    for i in range(len(FILTERS)):
        tw = np.asarray(inputs[f"tw{i}"], f32)[:, 0, 0, :]   # (64, S)
        tb = np.asarray(inputs[f"tb{i}"], f32)
        tg = np.asarray(inputs[f"tg{i}"], f32) * bn
        tbt = np.asarray(inputs[f"tbt{i}"], f32)
        tws.append(tg[:, None] * tw)
        tbs.append(tg * tb + tbt)

    fcw = np.asarray(inputs["fcw"], f32)                     # (4, 18528)
    fcb = np.asarray(inputs["fcb"], f32)
    rows, cols = np.tril_indices(N_FEAT, -1)                 # row-major
    Wdiag = fcw[:, :N_FEAT].copy()                           # (4, 192)
    Wtril = np.zeros((fcw.shape[0], N_FEAT, N_FEAT), f32)
    Wtril[:, rows, cols] = fcw[:, N_FEAT:]

    masks = np.zeros((N_FEAT, N_FEAT), f32)                  # mask_ge[gj, i]
    for gj in range(N_FEAT):
        masks[gj, gj:] = 1.0

    return A, c, tws, tbs, Wdiag, Wtril, fcb, masks


def _build(inputs):
    A, c, tws, tbs, Wdiag, Wtril, fcb, masks = _fold(inputs)
    eye = np.eye(N_FEAT, dtype=np.float32)

    def core_fn(Xs):
        """Xs: (SHARD, 32, 1000) fp32 -> (SHARD, 4)."""
        # duplicate each of the 32 channels twice -> o = 2g+j layout
        Xs2 = jnp.stack([Xs, Xs], 2).reshape(SHARD, 64, T)
        Xp = jnp.pad(Xs2, ((0, 0), (0, 0), (PADMAX, PADMAX)))
        Xt = []
        for f, size in enumerate(FILTERS):
            P = size // 2
            acc = jnp.broadcast_to(tbs[f][None, :, None], (SHARD, 64, T))
            for k in range(size):
                o = PADMAX - P + k
                acc = acc + tws[f][:, k][None, :, None] * Xp[:, :, o:o + T]
            Xt.append(acc)
        Xt = jnp.concatenate(Xt, 1)                           # (SHARD,192,T)
        G = jnp.einsum('bct,bdt->bcd', Xt, Xt, precision=HP) / (T - 1)

        # blocked Cholesky (lower), panels of NB columns, rank-2 steps
        # (scatter-free variants trip neuronx-cc bugs — keep .at[] form)
        Aw = G
        L = jnp.zeros_like(G)
        nblk = N_FEAT // NB
        for jb in range(nblk):
            j0 = jb * NB
            Pn = Aw[:, :, j0:j0 + NB]                         # (b,192,NB)
            colsl = []
            for j in range(0, NB, 2):
                gj = j0 + j
                d0 = jnp.sqrt(Pn[:, gj, j])
                col0 = (Pn[:, :, j] * (1.0 / d0)[:, None]
                        * masks[gj][None, :])
                c1 = Pn[:, :, j + 1] - col0 * col0[:, gj + 1][:, None]
                d1 = jnp.sqrt(c1[:, gj + 1])
                col1 = (c1 * (1.0 / d1)[:, None]
                        * masks[gj + 1][None, :])
                colsl += [col0, col1]
                CP = jnp.stack([col0, col1], -1)              # (b,192,2)
                R = jnp.stack([col0[:, j0:j0 + NB],
                               col1[:, j0:j0 + NB]], 1)       # (b,2,NB)
                Pn = Pn - jnp.einsum('bik,bkj->bij', CP, R, precision=HP)
            Lblk = jnp.stack(colsl, -1)                       # (b,192,NB)
            L = L.at[:, :, j0:j0 + NB].set(Lblk)
            if jb + 1 < nblk:
                upd = jnp.einsum('bik,bjk->bij', Lblk,
                                 Lblk[:, j0 + NB:, :], precision=HP)
                Aw = Aw.at[:, :, j0 + NB:].add(-upd)

        diag = jnp.sum(L * eye, -1)                           # (b,192)
        out = (jnp.log(diag) @ Wdiag.T
               + jnp.einsum('bij,kij->bk', L, Wtril, precision=HP)
               + fcb[None, :])
        return out

    if WIRE == "i8":
        def shard_fn(q, s):  # q: (32,SHARD,1000) int8, s: (32,SHARD) f32
            Xs = (q.astype(jnp.float32) * s[:, :, None]).transpose(1, 0, 2)
            return jax.lax.all_gather(core_fn(Xs), 'i')
    else:
        def shard_fn(q):     # q: (32,SHARD,1000) f16
            Xs = q.astype(jnp.float32).transpose(1, 0, 2)
            return jax.lax.all_gather(core_fn(Xs), 'i')

    return jax.pmap(shard_fn, axis_name='i'), A, c


_q_buf = np.empty((N_CORES, 32, SHARD, T), np.int8)
_s_buf = np.empty((N_CORES, 32, SHARD), np.float32)
_abs_buf = np.empty((32, SHARD, T), np.float32)
_f16_buf = np.empty((N_CORES, 32, SHARD, T), np.float16)


def _host_prep_i8(x, A, c):
    """Per-core (32,16,1000) int8 + (32,16) scales.  Single-threaded:
    measured faster than a thread pool on this 1-CPU host."""
    for i in range(N_CORES):
        xi = x[i * SHARD:(i + 1) * SHARD]                   # view (16,64,1000)
        xs = np.tensordot(A, xi, axes=([1], [1]))            # (32,16,1000)
        xs += c[:, None, None]
        np.abs(xs, out=_abs_buf)
        m = _abs_buf.max(axis=2)
        s = (m / 127.0 + 1e-30).astype(np.float32)
        _s_buf[i] = s
        xs *= (1.0 / s)[:, :, None]
        np.rint(xs, out=xs)
        _q_buf[i] = xs.astype(np.int8)
    return _q_buf, _s_buf


def _host_prep_f16(x, A, c):
    for i in range(N_CORES):
        xi = x[i * SHARD:(i + 1) * SHARD]
        xs = np.tensordot(A, xi, axes=([1], [1]))
        xs += c[:, None, None]
        _f16_buf[i] = xs.astype(np.float16)
    return _f16_buf


def kernel(**inputs):
    key = 'fn'
    if key not in _cache:
        _cache[key] = _build(inputs)
    fn, A, c = _cache[key]
    x = np.asarray(inputs["x"], np.float32)
    if WIRE == "i8":
        q, sc = _host_prep_i8(x, A, c)
        out = fn(q, sc)                                       # (8,8,16,4) replicated
    else:
        q = _host_prep_f16(x, A, c)
        out = fn(q)
    out0 = np.asarray(out.addressable_shards[0].data)         # one-device fetch
    return out0.reshape(B, -1).astype(np.float32)


if __name__ == "__main__":
    rng = np.random.default_rng(0)
    demo = {"x": rng.standard_normal((B, N_CHANS, T)).astype(np.float32)}
    print("self-test needs full inputs; run test.py instead")
